# revision 33
# baseline (speedup 1.0000x reference)
"""Trainium2 Bass kernel for nn_BaseKaolinRenderer (DIBR-style soft rasterizer).

Strategy
--------
Host (numpy, O(F) work, <1% of FLOPs): camera transform, face gather,
vertex-normal scatter-add, per-face linear edge/depth coefficients
  w_e(p) = Wx_e*px + Wy_e*py + Wc_e ,  z(p) = Zx*px + Zy*py + Zc
and y-band binning of faces (each face only influences pixels with
dmin >= -30*sigma = -0.3 barycentric, a small dilated triangle).

Device (8 NeuronCores, SPMD; core = batch*2 + half-image): per 128-pixel
band (2 image rows) x face-chunk (<=512):
  PE    : 4 matmuls [3,128]^T @ [3,n] -> w0,w1,w2,z in PSUM
  DVE   : min, min -> dmin; fused (z+pen, min-reduce) -> zm + running zmin;
          fused ((relu(1-e)+K)*C, mult-reduce) -> running prod of (1-c*prob);
          fused ((zm==zmin)*rev, add-reduce) -> argmin slot via rev-index sum
  ACT   : e = exp(100*dmin); r = relu(1-e)
  GPSIMD: pen = (dmin<0)*1e30
Host epilogue (O(P)): winning-face attribute interpolation, normal
normalization, improb = 1-prod, imdx.

The inputs are data-dependent only through band capacities, which are
computed at runtime inside kernel() before building/compiling the Bass
program (compilation happens per call; the NEFF is cached by content).
"""
import math
import numpy as np

H = W = 64
P = H * W
SIGMA = np.float32(0.01)
INV_SIGMA = 100.0
CREF = np.float32(np.float64(1.0) - 1e-7)          # 1 - 1e-7 in fp32
KADD = np.float32((1.0 - np.float64(CREF)) / np.float64(CREF))
DELTA = 0.3                                        # 30*sigma in barycentric units
BIG = np.float32(1e30)
COV_THR = np.float32(1e29)
ZINIT = 3.0e38
RBASE = 16384.0
ROWS_PER_BAND = 2
NBANDS = (H // 2) // ROWS_PER_BAND                 # 16 bands per half-image
CHUNK = 512


# ----------------------------------------------------------------- host math
def _preprocess(vertices, faces, attribs, camera_rot, camera_trans):
    B, V = vertices.shape[0], vertices.shape[1]
    vertices = vertices.astype(np.float32)
    camera_rot = camera_rot.astype(np.float32)
    camera_trans = camera_trans.astype(np.float32)
    faces = np.asarray(faces).astype(np.int64)
    vs_cam = (np.einsum('bvj,bij->bvi', vertices, camera_rot)
              + camera_trans[:, None, :]).astype(np.float32)
    fvc = vs_cam[:, faces]                                   # [B,F,3,3]
    vs_img = (vs_cam[..., :2] / vs_cam[..., 2:3]).astype(np.float32)
    fvi = vs_img[:, faces]                                   # [B,F,3,2]
    e1 = fvc[:, :, 1] - fvc[:, :, 0]
    e2 = fvc[:, :, 2] - fvc[:, :, 0]
    fnorm = np.cross(e1, e2).astype(np.float32)
    ids = faces.reshape(-1)
    data = np.repeat(fnorm, 3, axis=1)
    vnorm = np.zeros((B, V, 3), np.float32)
    for b in range(B):
        np.add.at(vnorm[b], ids, data[b])
    nrm = np.sqrt((vnorm ** 2).sum(-1, keepdims=True)).astype(np.float32)
    vnorm = (vnorm / (nrm + np.float32(1e-10))).astype(np.float32)
    fvn = vnorm[:, faces]
    face_attr = np.concatenate([fvn, attribs.astype(np.float32)], axis=-1)
    face_z = fvc[..., 2]
    v0, v1, v2 = fvi[:, :, 0], fvi[:, :, 1], fvi[:, :, 2]
    A = ((v1[..., 0] - v0[..., 0]) * (v2[..., 1] - v0[..., 1])
         - (v1[..., 1] - v0[..., 1]) * (v2[..., 0] - v0[..., 0])).astype(np.float32)
    A = np.where(np.abs(A) < np.float32(1e-8), np.float32(1e-8), A)
    return dict(fvi=fvi, face_z=face_z, face_attr=face_attr, A=A, faces=faces)


def _edge_coeffs(fvi, A, face_z):
    """W [3 edges, 3 (x,y,c), F] and Z [3 (x,y,c), F] fp32 (from f64)."""
    p = fvi.astype(np.float64)
    Ad = A.astype(np.float64)
    Fn = fvi.shape[0]
    Wc = np.empty((3, 3, Fn), np.float64)
    for e, (ia, ib) in enumerate(((1, 2), (2, 0), (0, 1))):
        ax, ay = p[:, ia, 0], p[:, ia, 1]
        bx, by = p[:, ib, 0], p[:, ib, 1]
        dx, dy = bx - ax, by - ay
        Wc[e, 0] = -dy / Ad
        Wc[e, 1] = dx / Ad
        Wc[e, 2] = (dy * ax - dx * ay) / Ad
    Z = np.einsum('ekf,fe->kf', Wc, face_z.astype(np.float64))
    return Wc.astype(np.float32), Z.astype(np.float32)


def _pixel_grid():
    xs = np.linspace(-1.0, 1.0, W, dtype=np.float32)
    ys = np.linspace(-1.0, 1.0, H, dtype=np.float32)
    return xs, ys


def _band_lists(fvi, A):
    """For one batch: per half -> list of (row0, face_idx_array) sorted by count desc."""
    ys3 = fvi[..., 1].astype(np.float64)
    s = ys3.sum(-1, keepdims=True)
    reg = (1 + 3 * DELTA) * ys3 - DELTA * s
    ymin, ymax = reg.min(-1), reg.max(-1)
    _, ypix = _pixel_grid()
    halves = []
    for h in range(2):
        bands = []
        for t in range(NBANDS):
            r0 = h * (H // 2) + t * ROWS_PER_BAND
            ylo, yhi = ypix[r0], ypix[r0 + ROWS_PER_BAND - 1]
            hit = np.nonzero((ymax >= ylo) & (ymin <= yhi))[0]
            bands.append((r0, hit))
        bands.sort(key=lambda x: -len(x[1]))
        halves.append(bands)
    return halves


# ------------------------------------------------------------- device kernel
def _build_nc(caps, repeat=1):
    import concourse.bass as bass
    import concourse.tile as tile
    from concourse import bacc, mybir

    nslots = len(caps)
    capsum = int(np.sum(caps))
    capmax = max(caps)

    nc = bacc.Bacc(None, target_bir_lowering=False, debug=False, num_devices=8)
    f32 = mybir.dt.float32
    Alu = mybir.AluOpType
    Act = mybir.ActivationFunctionType
    QBIAS = float(np.float32(1.0) - CREF)              # exact fp32 1-c

    pix_d = nc.dram_tensor("pix", [3, NBANDS * 128], f32, kind="ExternalInput").ap()
    coef_d = nc.dram_tensor("coef", [3, 3, capsum], f32, kind="ExternalInput").ap()
    out_d = nc.dram_tensor("out3", [128, 3 * NBANDS], f32, kind="ExternalOutput").ap()
    DCH = 2 * CHUNK                                    # coef DMA granularity

    with tile.TileContext(nc) as tc:
        with (
            tc.tile_pool(name="const", bufs=1) as cstp,
            tc.tile_pool(name="coef", bufs=3) as cpool,
            tc.tile_pool(name="psum", bufs=3, space="PSUM") as ppool,
            tc.tile_pool(name="psumz", bufs=2, space="PSUM") as pzpool,
            tc.tile_pool(name="scr", bufs=3) as spool,
            tc.tile_pool(name="zm", bufs=2) as zmpool,
            tc.tile_pool(name="acc", bufs=2) as apool,
        ):
            pix_s = cstp.tile([3, NBANDS * 128], f32)
            nc.sync.dma_start(pix_s[:], pix_d[:])
            revio = cstp.tile([128, capmax], f32)
            nc.gpsimd.iota(revio[:], pattern=[[-1, capmax]], base=int(RBASE),
                           channel_multiplier=0, allow_small_or_imprecise_dtypes=True)

            for _rep in range(repeat):
                sbase = 0
                for t in range(nslots):
                    cap = caps[t]
                    nft = math.ceil(cap / CHUNK)
                    lhsT = pix_s[:, t * 128:(t + 1) * 128]
                    zmst = zmpool.tile([128, capmax], f32, tag="zmst")
                    qst = zmpool.tile([128, capmax], f32, tag="qst")
                    # ---- stage 1: per-chunk compute (coef DMA per 2 chunks)
                    ct = None
                    for g in range(nft):
                        n = min(CHUNK, cap - g * CHUNK)
                        o0 = g * CHUNK
                        cof = o0 % DCH
                        if cof == 0:
                            nd = min(DCH, cap - o0)
                            ct = cpool.tile([3, 3, DCH], f32, tag="ct")
                            nc.sync.dma_start(ct[:, :, :nd],
                                              coef_d[:, :, sbase + o0:sbase + o0 + nd])
                        sp_ = ppool.tile([128, CHUNK], f32, tag="sp_")
                        dp_ = ppool.tile([128, CHUNK], f32, tag="dp_")
                        zp = pzpool.tile([128, CHUNK], f32, tag="zp")
                        nc.tensor.matmul(sp_[:, :n], lhsT, ct[:, 0, cof:cof + n], start=True, stop=True)
                        nc.tensor.matmul(dp_[:, :n], lhsT, ct[:, 1, cof:cof + n], start=True, stop=True)
                        nc.tensor.matmul(zp[:, :n], lhsT, ct[:, 2, cof:cof + n], start=True, stop=True)
                        ad = spool.tile([128, CHUNK], f32, tag="ad")
                        nc.scalar.activation(ad[:, :n], dp_[:, :n], Act.Abs)
                        w2s = spool.tile([128, CHUNK], f32, tag="w2s")
                        nc.scalar.activation(w2s[:, :n], sp_[:, :n], Act.Copy,
                                             bias=1.0, scale=-2.0)
                        m01 = spool.tile([128, CHUNK], f32, tag="m01")
                        nc.vector.tensor_tensor(m01[:, :n], sp_[:, :n], ad[:, :n],
                                                Alu.subtract)
                        dmin = spool.tile([128, CHUNK], f32, tag="dmin")
                        nc.vector.tensor_tensor(dmin[:, :n], m01[:, :n], w2s[:, :n], Alu.min)
                        pen = spool.tile([128, CHUNK], f32, tag="pen")
                        nc.gpsimd.tensor_scalar(pen[:, :n], dmin[:, :n], 0.0, float(BIG),
                                                Alu.is_lt, Alu.mult)
                        nc.vector.tensor_tensor(zmst[:, o0:o0 + n], zp[:, :n], pen[:, :n],
                                                Alu.add)
                        e = spool.tile([128, CHUNK], f32, tag="e")
                        nc.scalar.activation(e[:, :n], dmin[:, :n], Act.Exp, scale=INV_SIGMA)
                        t2 = spool.tile([128, CHUNK], f32, tag="t2")
                        nc.gpsimd.tensor_scalar(t2[:, :n], e[:, :n], -float(CREF),
                                                float(CREF), Alu.mult, Alu.add)
                        nc.gpsimd.tensor_scalar(qst[:, o0:o0 + n], t2[:, :n], 0.0,
                                                QBIAS, Alu.max, Alu.add)
                    # ---- stage 2: slot-wide reductions
                    oacc = apool.tile([128, 4], f32, tag="oacc")
                    nc.vector.tensor_reduce(oacc[:, 0:1], zmst[:, :cap],
                                            mybir.AxisListType.X, Alu.min)
                    # argmin slot via sum of (zm==zmin)*rev; in-place on zmst
                    nc.vector.scalar_tensor_tensor(
                        zmst[:, :cap], zmst[:, :cap], oacc[:, 0:1],
                        revio[:, :cap], Alu.is_equal, Alu.mult,
                        accum_out=oacc[:, 1:2])
                    # running product of q via mult-scan (zmst is free now)
                    nc.vector.tensor_tensor_scan(zmst[:, :cap], qst[:, :cap],
                                                 qst[:, :cap], 1.0,
                                                 Alu.mult, Alu.bypass)
                    nc.scalar.activation(oacc[:, 2:3], zmst[:, cap - 1:cap], Act.Copy)
                    nc.sync.dma_start(out_d[:, 3 * t:3 * t + 3], oacc[:, 0:3])
                    sbase += cap
    nc.compile()
    return nc


# ------------------------------------------------------------------ the API
def _prepare(vertices, faces, attribs, camera_rot, camera_trans):
    B = vertices.shape[0]
    pre = _preprocess(vertices, faces, attribs, camera_rot, camera_trans)

    # --- binning
    bands = [_band_lists(pre['fvi'][b], pre['A'][b]) for b in range(B)]
    counts = np.zeros(NBANDS, np.int64)
    for b in range(B):
        for h in range(2):
            for r, (_r0, idxs) in enumerate(bands[b][h]):
                counts[r] = max(counts[r], len(idxs))
    caps = [int(16 * math.ceil(c / 16)) for c in counts if c > 0]
    if not caps:
        caps = [16]
    nslots = len(caps)
    capsum = int(np.sum(caps))
    sbases = np.concatenate([[0], np.cumsum(caps)]).astype(np.int64)

    xs, ys = _pixel_grid()

    # --- per-core inputs
    in_maps = []
    slot2face = []
    for b in range(B):
        Wcf, Zcf = _edge_coeffs(pre['fvi'][b], pre['A'][b], pre['face_z'][b])
        Wd = Wcf.astype(np.float64)
        Scf = ((Wd[0] + Wd[1]) / 2).astype(np.float32)   # [3,F] coeffs of (w0+w1)/2
        Dcf = ((Wd[0] - Wd[1]) / 2).astype(np.float32)   # [3,F] coeffs of (w0-w1)/2
        for h in range(2):
            pix = np.zeros((3, NBANDS * 128), np.float32)
            coef = np.zeros((3, 3, capsum), np.float32)
            coef[2, 0] = -BIG                           # padding: S_c = -1e30 -> dmin=-1e30
            s2f = np.full(capsum, -1, np.int64)
            for r in range(nslots):
                r0, idxs = bands[b][h][r]
                colp = r * 128
                pxr = np.tile(xs, ROWS_PER_BAND)
                pyr = np.repeat(ys[r0:r0 + ROWS_PER_BAND], W)
                pix[0, colp:colp + 128] = pxr
                pix[1, colp:colp + 128] = pyr
                pix[2, colp:colp + 128] = 1.0
                m = len(idxs)
                if m:
                    s0 = sbases[r]
                    for k in range(3):
                        coef[k, 0, s0:s0 + m] = Scf[k, idxs]
                        coef[k, 1, s0:s0 + m] = Dcf[k, idxs]
                        coef[k, 2, s0:s0 + m] = Zcf[k, idxs]
                    s2f[s0:s0 + m] = idxs
            in_maps.append({"pix": pix, "coef": coef})
            slot2face.append(s2f)
    return dict(pre=pre, bands=bands, caps=caps, capsum=capsum, sbases=sbases,
                in_maps=in_maps, slot2face=slot2face, B=B)


def _epilogue(prep, results):
    pre, bands, slot2face = prep['pre'], prep['bands'], prep['slot2face']
    B, capsum, nslots = prep['B'], prep['capsum'], len(prep['caps'])
    sbases = prep['sbases']
    xs, ys = _pixel_grid()
    # --- host epilogue
    imnormal = np.zeros((B, H, W, 3), np.float32)
    im_attr_o = np.zeros((B, H, W, 3), np.float32)
    improb = np.zeros((B, H, W), np.float32)
    imdx = np.full((B, H, W), -1, np.int32)
    for b in range(B):
        fvi, face_z, face_attr, A = (pre['fvi'][b], pre['face_z'][b],
                                     pre['face_attr'][b], pre['A'][b])
        for h in range(2):
            core = b * 2 + h
            r_ = results[core]
            o3 = r_["out3"]                               # [128, 3*NBANDS]
            for r in range(nslots):
                r0, _idxs = bands[b][h][r]
                pz = o3[:, 3 * r]; pr = o3[:, 3 * r + 1]; pq = o3[:, 3 * r + 2]
                cov = pz < COV_THR
                bslot = np.clip(sbases[r] + (RBASE - pr).astype(np.int64),
                                0, capsum - 1)
                face = np.where(cov, slot2face[core][bslot], -1)
                ip = (np.float32(1.0) - pq).astype(np.float32)
                # pixel coords of this band
                pxr = np.tile(xs, ROWS_PER_BAND)
                pyr = np.repeat(ys[r0:r0 + ROWS_PER_BAND], W)
                # recompute winning barycentrics exactly like the reference
                f = np.maximum(face, 0)
                v = fvi[f]                                    # [128,3,2]
                a_e = np.empty((128, 3), np.float32)
                for e, (ia, ib) in enumerate(((1, 2), (2, 0), (0, 1))):
                    a_e[:, e] = ((v[:, ib, 0] - v[:, ia, 0]) * (pyr - v[:, ia, 1])
                                 - (v[:, ib, 1] - v[:, ia, 1]) * (pxr - v[:, ia, 0]))
                wb = (a_e / A[f][:, None]).astype(np.float32)
                attr_best = face_attr[f]                      # [128,3,6]
                ia_ = np.einsum('pk,pkc->pc', wb, attr_best).astype(np.float32)
                ia_ *= cov[:, None].astype(np.float32)
                nrm = np.sqrt((ia_[:, :3] ** 2).sum(-1, keepdims=True)).astype(np.float32)
                inorm = (ia_[:, :3] / (nrm + np.float32(1e-10))).astype(np.float32)
                rows = slice(r0, r0 + ROWS_PER_BAND)
                imnormal[b, rows] = inorm.reshape(ROWS_PER_BAND, W, 3)
                im_attr_o[b, rows] = ia_[:, 3:].reshape(ROWS_PER_BAND, W, 3)
                improb[b, rows] = ip.reshape(ROWS_PER_BAND, W)
                imdx[b, rows] = face.reshape(ROWS_PER_BAND, W).astype(np.int32)
    return (imnormal, im_attr_o), improb, imdx


def kernel(vertices, faces, attribs, camera_rot, camera_trans):
    from concourse.bass_utils import run_bass_kernel_spmd
    prep = _prepare(vertices, faces, attribs, camera_rot, camera_trans)
    nc = _build_nc(prep['caps'])
    res = run_bass_kernel_spmd(nc, prep['in_maps'], list(range(8)))
    return _epilogue(prep, res.results)


# revision 34
# speedup vs baseline: 1.0534x; 1.0534x over previous
"""Trainium2 Bass kernel for nn_BaseKaolinRenderer (DIBR-style soft rasterizer).

Strategy
--------
Host (numpy, O(F) work, <1% of FLOPs): camera transform, face gather,
vertex-normal scatter-add, per-face linear edge/depth coefficients
  w_e(p) = Wx_e*px + Wy_e*py + Wc_e ,  z(p) = Zx*px + Zy*py + Zc
and y-band binning of faces (each face only influences pixels with
dmin >= -30*sigma = -0.3 barycentric, a small dilated triangle).

Device (8 NeuronCores, SPMD; core = batch*2 + half-image): per 128-pixel
band (2 image rows) x face-chunk (<=512):
  PE    : 4 matmuls [3,128]^T @ [3,n] -> w0,w1,w2,z in PSUM
  DVE   : min, min -> dmin; fused (z+pen, min-reduce) -> zm + running zmin;
          fused ((relu(1-e)+K)*C, mult-reduce) -> running prod of (1-c*prob);
          fused ((zm==zmin)*rev, add-reduce) -> argmin slot via rev-index sum
  ACT   : e = exp(100*dmin); r = relu(1-e)
  GPSIMD: pen = (dmin<0)*1e30
Host epilogue (O(P)): winning-face attribute interpolation, normal
normalization, improb = 1-prod, imdx.

The inputs are data-dependent only through band capacities, which are
computed at runtime inside kernel() before building/compiling the Bass
program (compilation happens per call; the NEFF is cached by content).
"""
import math
import numpy as np

H = W = 64
P = H * W
SIGMA = np.float32(0.01)
INV_SIGMA = 100.0
CREF = np.float32(np.float64(1.0) - 1e-7)          # 1 - 1e-7 in fp32
KADD = np.float32((1.0 - np.float64(CREF)) / np.float64(CREF))
DELTA = 0.3                                        # 30*sigma in barycentric units
BIG = np.float32(1e30)
COV_THR = np.float32(1e29)
ZINIT = 3.0e38
RBASE = 16384.0
ROWS_PER_BAND = 2
NBANDS = (H // 2) // ROWS_PER_BAND                 # 16 bands per half-image
CHUNK = 512


# ----------------------------------------------------------------- host math
def _preprocess(vertices, faces, attribs, camera_rot, camera_trans):
    B, V = vertices.shape[0], vertices.shape[1]
    vertices = vertices.astype(np.float32)
    camera_rot = camera_rot.astype(np.float32)
    camera_trans = camera_trans.astype(np.float32)
    faces = np.asarray(faces).astype(np.int64)
    vs_cam = (np.einsum('bvj,bij->bvi', vertices, camera_rot)
              + camera_trans[:, None, :]).astype(np.float32)
    fvc = vs_cam[:, faces]                                   # [B,F,3,3]
    vs_img = (vs_cam[..., :2] / vs_cam[..., 2:3]).astype(np.float32)
    fvi = vs_img[:, faces]                                   # [B,F,3,2]
    e1 = fvc[:, :, 1] - fvc[:, :, 0]
    e2 = fvc[:, :, 2] - fvc[:, :, 0]
    fnorm = np.cross(e1, e2).astype(np.float32)
    ids = faces.reshape(-1)
    data = np.repeat(fnorm, 3, axis=1)
    vnorm = np.zeros((B, V, 3), np.float32)
    for b in range(B):
        np.add.at(vnorm[b], ids, data[b])
    nrm = np.sqrt((vnorm ** 2).sum(-1, keepdims=True)).astype(np.float32)
    vnorm = (vnorm / (nrm + np.float32(1e-10))).astype(np.float32)
    fvn = vnorm[:, faces]
    face_attr = np.concatenate([fvn, attribs.astype(np.float32)], axis=-1)
    face_z = fvc[..., 2]
    v0, v1, v2 = fvi[:, :, 0], fvi[:, :, 1], fvi[:, :, 2]
    A = ((v1[..., 0] - v0[..., 0]) * (v2[..., 1] - v0[..., 1])
         - (v1[..., 1] - v0[..., 1]) * (v2[..., 0] - v0[..., 0])).astype(np.float32)
    A = np.where(np.abs(A) < np.float32(1e-8), np.float32(1e-8), A)
    return dict(fvi=fvi, face_z=face_z, face_attr=face_attr, A=A, faces=faces)


def _edge_coeffs(fvi, A, face_z):
    """W [3 edges, 3 (x,y,c), F] and Z [3 (x,y,c), F] fp32 (from f64)."""
    p = fvi.astype(np.float64)
    Ad = A.astype(np.float64)
    Fn = fvi.shape[0]
    Wc = np.empty((3, 3, Fn), np.float64)
    for e, (ia, ib) in enumerate(((1, 2), (2, 0), (0, 1))):
        ax, ay = p[:, ia, 0], p[:, ia, 1]
        bx, by = p[:, ib, 0], p[:, ib, 1]
        dx, dy = bx - ax, by - ay
        Wc[e, 0] = -dy / Ad
        Wc[e, 1] = dx / Ad
        Wc[e, 2] = (dy * ax - dx * ay) / Ad
    Z = np.einsum('ekf,fe->kf', Wc, face_z.astype(np.float64))
    return Wc.astype(np.float32), Z.astype(np.float32)


def _pixel_grid():
    xs = np.linspace(-1.0, 1.0, W, dtype=np.float32)
    ys = np.linspace(-1.0, 1.0, H, dtype=np.float32)
    return xs, ys


def _band_lists(fvi, A):
    """For one batch: per half -> list of (row0, face_idx_array) sorted by count desc."""
    ys3 = fvi[..., 1].astype(np.float64)
    s = ys3.sum(-1, keepdims=True)
    reg = (1 + 3 * DELTA) * ys3 - DELTA * s
    ymin, ymax = reg.min(-1), reg.max(-1)
    _, ypix = _pixel_grid()
    halves = []
    for h in range(2):
        bands = []
        for t in range(NBANDS):
            r0 = h * (H // 2) + t * ROWS_PER_BAND
            ylo, yhi = ypix[r0], ypix[r0 + ROWS_PER_BAND - 1]
            hit = np.nonzero((ymax >= ylo) & (ymin <= yhi))[0]
            bands.append((r0, hit))
        bands.sort(key=lambda x: -len(x[1]))
        halves.append(bands)
    return halves


# ------------------------------------------------------------- device kernel
def _build_nc(caps, repeat=1):
    import concourse.bass as bass
    import concourse.tile as tile
    from concourse import bacc, mybir

    nslots = len(caps)
    capsum = int(np.sum(caps))
    capmax = max(caps)

    nc = bacc.Bacc(None, target_bir_lowering=False, debug=False, num_devices=8)
    f32 = mybir.dt.float32
    Alu = mybir.AluOpType
    Act = mybir.ActivationFunctionType
    QBIAS = float(np.float32(1.0) - CREF)              # exact fp32 1-c

    pix_d = nc.dram_tensor("pix", [3, NBANDS * 128], f32, kind="ExternalInput").ap()
    coef_d = nc.dram_tensor("coef", [3, 3, capsum], f32, kind="ExternalInput").ap()
    out_d = nc.dram_tensor("out3", [128, 3 * NBANDS], f32, kind="ExternalOutput").ap()
    DCH = 2 * CHUNK                                    # coef DMA granularity

    with tile.TileContext(nc) as tc:
        with (
            tc.tile_pool(name="const", bufs=1) as cstp,
            tc.tile_pool(name="coef", bufs=3) as cpool,
            tc.tile_pool(name="psum", bufs=2, space="PSUM") as ppool,
            tc.tile_pool(name="psumz", bufs=4, space="PSUM") as pzpool,
            tc.tile_pool(name="scr", bufs=3) as spool,
            tc.tile_pool(name="zm", bufs=2) as zmpool,
            tc.tile_pool(name="acc", bufs=2) as apool,
        ):
            pix_s = cstp.tile([3, NBANDS * 128], f32)
            nc.sync.dma_start(pix_s[:], pix_d[:])
            revio = cstp.tile([128, capmax], f32)
            nc.gpsimd.iota(revio[:], pattern=[[-1, capmax]], base=int(RBASE),
                           channel_multiplier=0, allow_small_or_imprecise_dtypes=True)

            for _rep in range(repeat):
                sbase = 0
                for t in range(nslots):
                    cap = caps[t]
                    nft = math.ceil(cap / CHUNK)
                    lhsT = pix_s[:, t * 128:(t + 1) * 128]
                    zmst = zmpool.tile([128, capmax], f32, tag="zmst")
                    qst = zmpool.tile([128, capmax], f32, tag="qst")
                    # ---- stage 1: per-chunk compute (coef DMA per 2 chunks)
                    ct = None
                    for g in range(nft):
                        n = min(CHUNK, cap - g * CHUNK)
                        o0 = g * CHUNK
                        cof = o0 % DCH
                        if cof == 0:
                            nd = min(DCH, cap - o0)
                            ct = cpool.tile([3, 3, DCH], f32, tag="ct")
                            nc.sync.dma_start(ct[:, :, :nd],
                                              coef_d[:, :, sbase + o0:sbase + o0 + nd])
                        sp_ = ppool.tile([128, CHUNK], f32, tag="sp_")
                        dp_ = ppool.tile([128, CHUNK], f32, tag="dp_")
                        zp = pzpool.tile([128, CHUNK], f32, tag="zp")
                        nc.tensor.matmul(sp_[:, :n], lhsT, ct[:, 0, cof:cof + n], start=True, stop=True)
                        nc.tensor.matmul(dp_[:, :n], lhsT, ct[:, 1, cof:cof + n], start=True, stop=True)
                        nc.tensor.matmul(zp[:, :n], lhsT, ct[:, 2, cof:cof + n], start=True, stop=True)
                        ad = spool.tile([128, CHUNK], f32, tag="ad")
                        nc.scalar.activation(ad[:, :n], dp_[:, :n], Act.Abs)
                        w2s = spool.tile([128, CHUNK], f32, tag="w2s")
                        nc.scalar.activation(w2s[:, :n], sp_[:, :n], Act.Copy,
                                             bias=1.0, scale=-2.0)
                        m01 = spool.tile([128, CHUNK], f32, tag="m01")
                        nc.vector.tensor_tensor(m01[:, :n], sp_[:, :n], ad[:, :n],
                                                Alu.subtract)
                        dmin = spool.tile([128, CHUNK], f32, tag="dmin")
                        nc.vector.tensor_tensor(dmin[:, :n], m01[:, :n], w2s[:, :n], Alu.min)
                        pen = spool.tile([128, CHUNK], f32, tag="pen")
                        nc.gpsimd.tensor_scalar(pen[:, :n], dmin[:, :n], 0.0, float(BIG),
                                                Alu.is_lt, Alu.mult)
                        nc.vector.tensor_tensor(zmst[:, o0:o0 + n], zp[:, :n], pen[:, :n],
                                                Alu.add)
                        e = spool.tile([128, CHUNK], f32, tag="e")
                        nc.scalar.activation(e[:, :n], dmin[:, :n], Act.Exp, scale=INV_SIGMA)
                        t2 = spool.tile([128, CHUNK], f32, tag="t2")
                        nc.gpsimd.tensor_scalar(t2[:, :n], e[:, :n], -float(CREF),
                                                float(CREF), Alu.mult, Alu.add)
                        nc.gpsimd.tensor_scalar(qst[:, o0:o0 + n], t2[:, :n], 0.0,
                                                QBIAS, Alu.max, Alu.add)
                    # ---- stage 2: slot-wide reductions
                    oacc = apool.tile([128, 4], f32, tag="oacc")
                    nc.vector.tensor_reduce(oacc[:, 0:1], zmst[:, :cap],
                                            mybir.AxisListType.X, Alu.min)
                    # argmin slot via sum of (zm==zmin)*rev; in-place on zmst
                    nc.vector.scalar_tensor_tensor(
                        zmst[:, :cap], zmst[:, :cap], oacc[:, 0:1],
                        revio[:, :cap], Alu.is_equal, Alu.mult,
                        accum_out=oacc[:, 1:2])
                    # running product of q via mult-scan (zmst is free now)
                    nc.vector.tensor_tensor_scan(zmst[:, :cap], qst[:, :cap],
                                                 qst[:, :cap], 1.0,
                                                 Alu.mult, Alu.bypass)
                    nc.scalar.activation(oacc[:, 2:3], zmst[:, cap - 1:cap], Act.Copy)
                    nc.sync.dma_start(out_d[:, 3 * t:3 * t + 3], oacc[:, 0:3])
                    sbase += cap
    nc.compile()
    return nc


# ------------------------------------------------------------------ the API
def _prepare(vertices, faces, attribs, camera_rot, camera_trans):
    B = vertices.shape[0]
    pre = _preprocess(vertices, faces, attribs, camera_rot, camera_trans)

    # --- binning
    bands = [_band_lists(pre['fvi'][b], pre['A'][b]) for b in range(B)]
    counts = np.zeros(NBANDS, np.int64)
    for b in range(B):
        for h in range(2):
            for r, (_r0, idxs) in enumerate(bands[b][h]):
                counts[r] = max(counts[r], len(idxs))
    caps = [int(16 * math.ceil(c / 16)) for c in counts if c > 0]
    if not caps:
        caps = [16]
    nslots = len(caps)
    capsum = int(np.sum(caps))
    sbases = np.concatenate([[0], np.cumsum(caps)]).astype(np.int64)

    xs, ys = _pixel_grid()

    # --- per-core inputs
    in_maps = []
    slot2face = []
    for b in range(B):
        Wcf, Zcf = _edge_coeffs(pre['fvi'][b], pre['A'][b], pre['face_z'][b])
        Wd = Wcf.astype(np.float64)
        Scf = ((Wd[0] + Wd[1]) / 2).astype(np.float32)   # [3,F] coeffs of (w0+w1)/2
        Dcf = ((Wd[0] - Wd[1]) / 2).astype(np.float32)   # [3,F] coeffs of (w0-w1)/2
        for h in range(2):
            pix = np.zeros((3, NBANDS * 128), np.float32)
            coef = np.zeros((3, 3, capsum), np.float32)
            coef[2, 0] = -BIG                           # padding: S_c = -1e30 -> dmin=-1e30
            s2f = np.full(capsum, -1, np.int64)
            for r in range(nslots):
                r0, idxs = bands[b][h][r]
                colp = r * 128
                pxr = np.tile(xs, ROWS_PER_BAND)
                pyr = np.repeat(ys[r0:r0 + ROWS_PER_BAND], W)
                pix[0, colp:colp + 128] = pxr
                pix[1, colp:colp + 128] = pyr
                pix[2, colp:colp + 128] = 1.0
                m = len(idxs)
                if m:
                    s0 = sbases[r]
                    for k in range(3):
                        coef[k, 0, s0:s0 + m] = Scf[k, idxs]
                        coef[k, 1, s0:s0 + m] = Dcf[k, idxs]
                        coef[k, 2, s0:s0 + m] = Zcf[k, idxs]
                    s2f[s0:s0 + m] = idxs
            in_maps.append({"pix": pix, "coef": coef})
            slot2face.append(s2f)
    return dict(pre=pre, bands=bands, caps=caps, capsum=capsum, sbases=sbases,
                in_maps=in_maps, slot2face=slot2face, B=B)


def _epilogue(prep, results):
    pre, bands, slot2face = prep['pre'], prep['bands'], prep['slot2face']
    B, capsum, nslots = prep['B'], prep['capsum'], len(prep['caps'])
    sbases = prep['sbases']
    xs, ys = _pixel_grid()
    # --- host epilogue
    imnormal = np.zeros((B, H, W, 3), np.float32)
    im_attr_o = np.zeros((B, H, W, 3), np.float32)
    improb = np.zeros((B, H, W), np.float32)
    imdx = np.full((B, H, W), -1, np.int32)
    for b in range(B):
        fvi, face_z, face_attr, A = (pre['fvi'][b], pre['face_z'][b],
                                     pre['face_attr'][b], pre['A'][b])
        for h in range(2):
            core = b * 2 + h
            r_ = results[core]
            o3 = r_["out3"]                               # [128, 3*NBANDS]
            for r in range(nslots):
                r0, _idxs = bands[b][h][r]
                pz = o3[:, 3 * r]; pr = o3[:, 3 * r + 1]; pq = o3[:, 3 * r + 2]
                cov = pz < COV_THR
                bslot = np.clip(sbases[r] + (RBASE - pr).astype(np.int64),
                                0, capsum - 1)
                face = np.where(cov, slot2face[core][bslot], -1)
                ip = (np.float32(1.0) - pq).astype(np.float32)
                # pixel coords of this band
                pxr = np.tile(xs, ROWS_PER_BAND)
                pyr = np.repeat(ys[r0:r0 + ROWS_PER_BAND], W)
                # recompute winning barycentrics exactly like the reference
                f = np.maximum(face, 0)
                v = fvi[f]                                    # [128,3,2]
                a_e = np.empty((128, 3), np.float32)
                for e, (ia, ib) in enumerate(((1, 2), (2, 0), (0, 1))):
                    a_e[:, e] = ((v[:, ib, 0] - v[:, ia, 0]) * (pyr - v[:, ia, 1])
                                 - (v[:, ib, 1] - v[:, ia, 1]) * (pxr - v[:, ia, 0]))
                wb = (a_e / A[f][:, None]).astype(np.float32)
                attr_best = face_attr[f]                      # [128,3,6]
                ia_ = np.einsum('pk,pkc->pc', wb, attr_best).astype(np.float32)
                ia_ *= cov[:, None].astype(np.float32)
                nrm = np.sqrt((ia_[:, :3] ** 2).sum(-1, keepdims=True)).astype(np.float32)
                inorm = (ia_[:, :3] / (nrm + np.float32(1e-10))).astype(np.float32)
                rows = slice(r0, r0 + ROWS_PER_BAND)
                imnormal[b, rows] = inorm.reshape(ROWS_PER_BAND, W, 3)
                im_attr_o[b, rows] = ia_[:, 3:].reshape(ROWS_PER_BAND, W, 3)
                improb[b, rows] = ip.reshape(ROWS_PER_BAND, W)
                imdx[b, rows] = face.reshape(ROWS_PER_BAND, W).astype(np.int32)
    return (imnormal, im_attr_o), improb, imdx


def kernel(vertices, faces, attribs, camera_rot, camera_trans):
    from concourse.bass_utils import run_bass_kernel_spmd
    prep = _prepare(vertices, faces, attribs, camera_rot, camera_trans)
    nc = _build_nc(prep['caps'])
    res = run_bass_kernel_spmd(nc, prep['in_maps'], list(range(8)))
    return _epilogue(prep, res.results)
